# revision 1
# baseline (speedup 1.0000x reference)
"""Trainium2 Bass kernel for nn_AAttn (area-attention block).

Reference computation (per batch image [C=256, 64, 64]):
  qkv  = BN(1x1 conv, 3C)          -> split per head h: q,k,v (hd=32)
  area attention over 4 areas of 1024 px each (16 rows x 64 cols)
  o    = softmax(q^T k / sqrt(hd)) @ v
  pe   = BN(7x7 depthwise conv(v_map))
  out  = BN(1x1 conv(o + pe))

Sharding: fully data-parallel. 8 cores x (1 batch-half = 2 areas = 32 rows).
Each core gets a 38-row halo slab of x; everything else is computed locally
(halo of v for the depthwise conv is recomputed from the x halo). No
collectives.

Device layout decisions:
  - qkv weights are host-permuted to head-major [Q(256); K(256); V(256)]
    rows, BN scale folded into W, attention scale folded into Q rows.
  - S^T orientation: S^T[m, n] = sum_d k[d,m] q[d,n]; 4 heads run
    concurrently on PE 32x128 row-tiles; exp on ScalarE (no max-sub needed,
    logits are O(1)); O and denominator accumulate via 128x32 col-tiles.
  - depthwise conv: 49 fused multiply-accumulate taps on VectorE
    (scalar_tensor_tensor), initialized with the attention output so the
    add is free.
  - all BN biases are folded on host; v-bias enters attention output via
    the softmax-sums-to-one identity, pe/proj biases fold into one vector.
"""

import os
import sys

os.environ.setdefault("MYCRO_LOCAL_CACHE", "1")
if "/opt/trn_rl_repo" not in sys.path:
    sys.path.insert(0, "/opt/trn_rl_repo")

from contextlib import ExitStack

import ml_dtypes
import numpy as np

import concourse.bass as bass
import concourse.bacc as bacc
import concourse.tile as tile
from concourse import mybir
from concourse.bass_utils import run_bass_kernel_spmd


def _install_ntff_hook_shim():
    """The agent image's antenv lacks axon_hooks; recreate it so
    run_bass_kernel_spmd(trace=True) can NTFF-profile via the axon .so."""
    import types
    try:
        from antenv.axon_hooks import get_axon_ntff_profile_hook  # noqa: F401
        return  # real module exists
    except ImportError:
        pass
    try:
        from trn_agent_boot.trn_boot import _ntff_profile_via_ctypes
        hook = _ntff_profile_via_ctypes("/opt/axon/libaxon_pjrt.so")
    except Exception:
        hook = None
    mod = types.ModuleType("antenv.axon_hooks")
    _state = {"hook": hook}
    mod.get_axon_ntff_profile_hook = lambda: _state["hook"]
    mod.set_axon_ntff_profile_hook = lambda h: _state.update(hook=h)
    sys.modules["antenv.axon_hooks"] = mod
    import antenv
    antenv.axon_hooks = mod


_install_ntff_hook_shim()

F32 = mybir.dt.float32
BF16 = mybir.dt.bfloat16
BF16NP = ml_dtypes.bfloat16

B, C, H, W = 4, 256, 64, 64
NH, HD, AREA = 8, 32, 4
EPS = 1e-5
NCORES = 8

CORE_ROWS = 32          # image rows per core
HALO = 3                # 7x7 conv halo
SLAB_ROWS = CORE_ROWS + 2 * HALO   # 38
PX = SLAB_ROWS * W      # 2432 slab pixels
CPX = CORE_ROWS * W     # 2048 core pixels
PXOFF = HALO * W        # 192: slab px offset of core region
NA = 1024               # pixels per area

LAST_EXEC_NS = [None]
LAST_RESULTS = [None]

# 8 conv taps per ctile on VectorE (interleaved after normalizes), the
# other 41 as PE diagonal matmuls
DVE_TAPS = [(dy, 0) for dy in range(-3, 4)] + [(0, -1)]
PE_TAPS = [(dy, dx) for dy in range(-3, 4) for dx in range(-3, 4)
           if (dy, dx) not in DVE_TAPS]
NPE = len(PE_TAPS)


def _build_graph():
    nc = bacc.Bacc()

    x_ext = nc.declare_dram_parameter("x", [C, PX], BF16, isOutput=False)
    vmask_ext = nc.declare_dram_parameter("vmask", [1, PX], BF16, isOutput=False)
    wqkv_ext = nc.declare_dram_parameter("wqkv", [C, 3 * C], BF16, isOutput=False)
    bqkv_ext = nc.declare_dram_parameter("bqkv", [3 * C, 1], F32, isOutput=False)
    bqkvr_ext = nc.declare_dram_parameter("bqkvr", [1, 3 * C], BF16, isOutput=False)
    wpe_ext = nc.declare_dram_parameter("wpe", [C, 49], F32, isOutput=False)
    wdiag_ext = nc.declare_dram_parameter("wdiag", [128, 2 * NPE * 128], BF16,
                                          isOutput=False)
    wproj_ext = nc.declare_dram_parameter("wproj", [C, C], BF16, isOutput=False)
    btot_ext = nc.declare_dram_parameter("btot", [C, 1], F32, isOutput=False)
    out_ext = nc.declare_dram_parameter("out", [C, CPX], F32, isOutput=True)

    VROW = W + 6           # v row pitch: 3 zero cols each side
    VP = VROW * SLAB_ROWS  # 2660

    with tile.TileContext(nc) as tc, ExitStack() as ctx:
        persist = ctx.enter_context(tc.tile_pool(name="persist", bufs=1))
        e_pool = ctx.enter_context(tc.tile_pool(name="epool", bufs=12))
        wk_pool = ctx.enter_context(tc.tile_pool(name="wkpool", bufs=2))
        mm_ctx = tc.tile_pool(name="mmps", bufs=3, space="PSUM")
        mm_ps = mm_ctx.__enter__()

        def ptile(shape, dtype, name):
            return persist.tile(shape, dtype, name=name, tag=name)


        # ---------------- persistent SBUF tensors ----------------
        wp_t = [ptile([128, C], BF16, name=f"wp{k}") for k in range(2)]
        wpe_t = [ptile([128, 49], F32, name=f"wpe{k}") for k in range(2)]
        btot_t = [ptile([128, 1], F32, name=f"btot{m}") for m in range(2)]
        wdiag_t = ptile([128, 2 * NPE * 128], BF16, name="wdiag")
        ones_t = ptile([128, 32], BF16, name="ones")

        q_sb = [ptile([128, CPX], BF16, name=f"q{h}") for h in range(2)]
        k_sb = [ptile([128, CPX], BF16, name=f"k{h}") for h in range(2)]
        v_sb = [ptile([128, VP], BF16, name=f"v{cti}") for cti in range(2)]
        vt_sb = [ptile([128, 2048], BF16, name=f"vt{a}") for a in range(2)]
        onorm_sb = [ptile([128, CPX], BF16, name=f"onorm{h}") for h in range(2)]
        acc_sb = [ptile([128, CPX], BF16, name=f"acc{cti}") for cti in range(2)]
        pin_sb = [ptile([128, CPX], BF16, name=f"pin{cti}") for cti in range(2)]
        out_sb = [ptile([128, CPX], F32, name=f"outsb{cti}") for cti in range(2)]

        # early (released before attention): x, qkv weights, mask
        x_t = [ptile([128, PX], BF16, name=f"x{k}") for k in range(2)]
        wq_t = [ptile([128, 3 * C], BF16, name=f"wq{k}") for k in range(2)]
        bias_t = [ptile([128, 1], F32, name=f"bias{m}") for m in range(4, 6)]
        biasr_t = ptile([1, 3 * C], BF16, name="biasr")
        onesr_t = ptile([1, PX], BF16, name="onesr")
        maskr_t = ptile([1, PX], BF16, name="maskr")
        mask_t = ptile([128, PX], BF16, name="mask")

        # ---------------- input DMAs ----------------
        for k in range(2):
            nc.sync.dma_start(x_t[k][:], x_ext[128 * k:128 * (k + 1), :])
            nc.sync.dma_start(wq_t[k][:], wqkv_ext[128 * k:128 * (k + 1), :])
            nc.sync.dma_start(wp_t[k][:], wproj_ext[128 * k:128 * (k + 1), :])
            nc.sync.dma_start(wpe_t[k][:], wpe_ext[128 * k:128 * (k + 1), :])
            nc.sync.dma_start(btot_t[k][:], btot_ext[128 * k:128 * (k + 1), :])
        for m in (4, 5):
            nc.sync.dma_start(bias_t[m - 4][:], bqkv_ext[128 * m:128 * (m + 1), :])
        nc.sync.dma_start(biasr_t[:], bqkvr_ext[:])
        nc.sync.dma_start(maskr_t[:], vmask_ext[:])
        nc.sync.dma_start(wdiag_t[:], wdiag_ext[:])
        nc.gpsimd.partition_broadcast(mask_t[:], maskr_t[:])
        nc.vector.memset(ones_t[:], 1.0)
        nc.vector.memset(onesr_t[:], 1.0)
        for k in range(2):
            # zero everything; evacs fill the 64-wide data blocks of each row
            nc.gpsimd.memset(v_sb[k][:], 0.0)
            nc.gpsimd.memset(acc_sb[k][:], 0.0)

        # ---------------- qkv 1x1 conv (matmul) + BN ----------------
        # Q/K: bias folded in via a K=1 ones-row matmul; psum tiles of
        # [128,1024] so each evacuation move is big; evacs alternate
        # ScalarE/VectorE. V: bias+mask fused in the stt evac on VectorE.
        for mc in (0, 2):  # only head-set 0 Q/K before attention starts
            is_v = mc >= 4
            npx = PX if is_v else CPX
            off = 0 if is_v else PXOFF
            pcs = [(i * 1024, min(1024, npx - i * 1024))
                   for i in range((npx + 1023) // 1024)]
            for ti_, (pco, pcn) in enumerate(pcs):
                ps = mm_ps.tile([128, 1024], F32, tag="mm")
                for half in range(0, pcn, 512):
                    hn = min(512, pcn - half)
                    for kc in range(2):
                        nc.tensor.matmul(
                            ps[:, half:half + hn],
                            lhsT=wq_t[kc][:, 128 * mc:128 * (mc + 1)],
                            rhs=x_t[kc][:, off + pco + half:off + pco + half + hn],
                            start=(kc == 0), stop=(kc == 1 and is_v),
                        )
                    if not is_v:
                        nc.tensor.matmul(
                            ps[:, half:half + hn],
                            lhsT=biasr_t[:, 128 * mc:128 * (mc + 1)],
                            rhs=onesr_t[:, 0:hn],
                            start=False, stop=True,
                        )
                if is_v:
                    r0, nr = pco // W, pcn // W
                    v70 = v_sb[mc - 4][:].rearrange("p (r c) -> p r c", c=VROW)
                    nc.vector.scalar_tensor_tensor(
                        out=v70[:, r0:r0 + nr, 3:3 + W],
                        in0=ps[:, :pcn], scalar=bias_t[mc - 4],
                        in1=mask_t[:, pco:pco + pcn],
                        op0=mybir.AluOpType.add, op1=mybir.AluOpType.mult,
                    )
                else:
                    dst = q_sb[mc] if mc < 2 else k_sb[mc - 2]
                    if ti_ % 2 == 0:
                        nc.scalar.activation(
                            dst[:, pco:pco + pcn], ps[:, :pcn],
                            mybir.ActivationFunctionType.Copy)
                    else:
                        nc.vector.tensor_copy(dst[:, pco:pco + pcn], ps[:, :pcn])

        mm_ctx.__exit__(None, None, None)
        s_ps_ctx = tc.tile_pool(name="sps", bufs=1, space="PSUM")
        s_ps_pool = s_ps_ctx.__enter__()
        s2_ps_ctx = tc.tile_pool(name="sps2", bufs=1, space="PSUM")
        s2_ps_pool = s2_ps_ctx.__enter__()
        od_ctx = tc.tile_pool(name="odps", bufs=3, space="PSUM")
        od_pool = od_ctx.__enter__()

        # ------------- attention (phase-split) + PE-side conv -------------
        def qkv_tail_piece(od_pool_, kind, arg):
            if kind == "qk":
                mc, pco = arg
                pcn = 512
                ps = od_pool_.tile([128, 512], F32, tag="od")
                for kc in range(2):
                    nc.tensor.matmul(
                        ps[:, :pcn],
                        lhsT=wq_t[kc][:, 128 * mc:128 * (mc + 1)],
                        rhs=x_t[kc][:, PXOFF + pco:PXOFF + pco + pcn],
                        start=(kc == 0), stop=False,
                    )
                nc.tensor.matmul(
                    ps[:, :pcn],
                    lhsT=biasr_t[:, 128 * mc:128 * (mc + 1)],
                    rhs=onesr_t[:, 0:pcn],
                    start=False, stop=True,
                )
                dst = q_sb[1] if mc == 1 else k_sb[1]
                nc.vector.tensor_copy(dst[:, pco:pco + pcn], ps[:, :pcn])
            elif kind == "v":
                mc, pco = arg
                pcn = min(512, PX - pco)
                ps = od_pool_.tile([128, 512], F32, tag="od")
                for kc in range(2):
                    nc.tensor.matmul(
                        ps[:, :pcn],
                        lhsT=wq_t[kc][:, 128 * mc:128 * (mc + 1)],
                        rhs=x_t[kc][:, pco:pco + pcn],
                        start=(kc == 0), stop=(kc == 1),
                    )
                r0, nr = pco // W, (pcn + W - 1) // W
                v70 = v_sb[mc - 4][:].rearrange("p (r c) -> p r c", c=VROW)
                nc.vector.scalar_tensor_tensor(
                    out=v70[:, r0:r0 + nr, 3:3 + W],
                    in0=ps[:, :pcn], scalar=bias_t[mc - 4],
                    in1=mask_t[:, pco:pco + pcn],
                    op0=mybir.AluOpType.add, op1=mybir.AluOpType.mult,
                )
            else:  # vT
                a, g = arg
                ps = od_pool_.tile([128, 512], F32, tag="od")
                for jj in range(2):
                    j = 2 * g + jj
                    pxo = PXOFF + NA * a + 128 * j
                    for kc in range(2):
                        nc.tensor.matmul(
                            ps[:, 256 * jj:256 * (jj + 1)],
                            lhsT=x_t[kc][:, pxo:pxo + 128],
                            rhs=wq_t[kc][:, 2 * C:3 * C],
                            start=(kc == 0), stop=(kc == 1),
                        )
                nc.vector.tensor_copy(vt_sb[a][:, 512 * g:512 * (g + 1)],
                                      ps[:, 0:512])

        def corrections_and_combine(cti):
            nc.vector.tensor_add(pin_sb[cti][:], pin_sb[cti][:],
                                 acc_sb[cti][:])
            nc.vector.tensor_add(pin_sb[cti][:], pin_sb[cti][:],
                                 onorm_sb[cti][:])

        def emit_dve_taps(cti, t0, t1):
            acc = acc_sb[cti]
            a3 = acc[:].rearrange("p (r c) -> p r c", c=W)
            v70 = v_sb[cti][:].rearrange("p (r c) -> p r c", c=VROW)
            for ti in range(t0, min(t1, len(DVE_TAPS))):
                dy, dx = DVE_TAPS[ti]
                tcol = (dy + 3) * 7 + (dx + 3)
                nc.vector.scalar_tensor_tensor(
                    out=a3[:, :, :],
                    in0=v70[:, HALO + dy:HALO + dy + CORE_ROWS,
                            3 + dx:3 + dx + W],
                    scalar=wpe_t[cti][:, tcol:tcol + 1],
                    in1=a3[:, :, :],
                    op0=mybir.AluOpType.mult,
                    op1=mybir.AluOpType.add,
                )

        def conv_chunk(ci):
            cti, c = ci // 4, ci % 4
            ps = od_pool.tile([128, 512], F32, tag="od")
            v70 = v_sb[cti][:].rearrange("p (r c) -> p r c", c=VROW)
            for ti, (dy, dx) in enumerate(PE_TAPS):
                dcol = 128 * (NPE * cti + ti)
                r0 = HALO + dy + 8 * c
                nc.tensor.matmul(
                    ps[:], lhsT=wdiag_t[:, dcol:dcol + 128],
                    rhs=v70[:, r0:r0 + 8, 3 + dx:3 + dx + W],
                    start=(ti == 0), stop=(ti == len(PE_TAPS) - 1),
                    skip_group_check=True,
                )
            nc.vector.tensor_copy(
                pin_sb[cti][:, 512 * c:512 * (c + 1)], ps[:])

        # i=0 must cover: Q1/K1 (read by A(1)), vT0 (B(0)), all of V0
        # (ct0 taps at i=0's normalize). i=1: vT1 (B(2)) and V1 (ct1 taps).
        tail_work = (
            [("qk", (1, 512 * t)) for t in range(4)]
            + [("qk", (3, 512 * t)) for t in range(4)]
            + [("vT", (0, g)) for g in range(4)]
            + [("v", (4, 512 * t)) for t in range(5)]
            + [("vT", (1, g)) for g in range(4)]
            + [("v", (5, 512 * t)) for t in range(5)]
        )
        tail_sched = {0: 12, 1: 9, 2: 5}

        its = [(hs, a, nu) for hs in range(2) for a in range(2) for nu in range(2)]
        for inum, (hs, a, nu) in enumerate(its):
            no = NA * a + 512 * nu   # n offset in core px
            units = [(j, hp) for j in range(8) for hp in range(4)]
            # ---- phase A: S + exp -> E tiles ----
            e_tiles = []
            gi = 0
            galt = 0
            while gi < len(units):
                gsz = 3 if galt % 2 == 0 else 2
                galt += 1
                grp = units[gi:gi + gsz]
                gi += len(grp)
                if gsz == 3:
                    s_ps = s_ps_pool.tile([128, 1536], F32, tag="s")
                else:
                    s_ps = s2_ps_pool.tile([128, 1024], F32, tag="s2")
                for idx, (j, hp) in enumerate(grp):
                    nc.tensor.matmul(
                        s_ps[:, 512 * idx:512 * (idx + 1)],
                        lhsT=k_sb[hs][32 * hp:32 * (hp + 1),
                                      NA * a + 128 * j:NA * a + 128 * (j + 1)],
                        rhs=q_sb[hs][32 * hp:32 * (hp + 1), no:no + 512],
                        start=True, stop=True,
                        tile_position=(32 * hp, 0),
                    )
                ncols = 512 * len(grp)
                e_t = e_pool.tile([128, 512 * gsz], BF16,
                                  tag="e" if gsz == 3 else "e2", bufs=12)
                nc.scalar.activation(
                    e_t[:, :ncols], s_ps[:, :ncols],
                    mybir.ActivationFunctionType.Exp)
                e_tiles.append((grp, e_t))
            # conv burst for the previous iteration: sits between this
            # iteration's S production and its O/den consumption on the PE
            # queue, so ScalarE always has S data to exp.
            nwork = tail_sched.get(inum, 0)
            for _ in range(nwork):
                if tail_work:
                    qkv_tail_piece(od_pool, *tail_work.pop(0))
            if inum > 0:
                conv_chunk(inum - 1)
            if inum == len(its) - 1:
                conv_chunk(inum)
            if inum - 1 == 4:
                corrections_and_combine(0)
            # ---- phase B: dense O + den burst ----
            o_ps = od_pool.tile([128, 512], F32, tag="od")
            den_ps = od_pool.tile([128, 512], F32, tag="od")
            for grp, e_t in e_tiles:
                for idx, (j, hp) in enumerate(grp):
                    first, last = (j == 0), (j == 7)
                    nc.tensor.matmul(
                        o_ps[32 * hp:32 * (hp + 1), :],
                        lhsT=vt_sb[a][:, 256 * j + 32 * (4 * hs + hp):
                                       256 * j + 32 * (4 * hs + hp + 1)],
                        rhs=e_t[:, 512 * idx:512 * (idx + 1)],
                        start=first, stop=last,
                        skip_group_check=True,
                        tile_position=(0, 32 * hp),
                    )
                    nc.tensor.matmul(
                        den_ps[32 * hp:32 * (hp + 1), :],
                        lhsT=ones_t[:, 0:32],
                        rhs=e_t[:, 512 * idx:512 * (idx + 1)],
                        start=first, stop=last,
                        skip_group_check=True,
                        tile_position=(0, 32 * hp),
                    )
            rd32 = wk_pool.tile([128, 512], F32, tag="rd32")
            nc.vector.reciprocal_approx_fast(rd32[:], den_ps[:])
            nc.vector.tensor_mul(
                onorm_sb[hs][:, no:no + 512], o_ps[:], rd32[:])
            if 1 <= inum <= 4:
                emit_dve_taps(0, 2 * (inum - 1), 2 * inum)
            elif inum >= 5:
                emit_dve_taps(1, 3 * (inum - 5), 3 * (inum - 5) + 3)
            if inum == len(its) - 1:
                corrections_and_combine(1)

        od_ctx.__exit__(None, None, None)
        s2_ps_ctx.__exit__(None, None, None)
        s_ps_ctx.__exit__(None, None, None)

        # ---------------- proj 1x1 conv + BN ----------------
        pr_ctx = tc.tile_pool(name="prps", bufs=2, space="PSUM")
        pr_ps = pr_ctx.__enter__()
        for mc in range(2):
            for pc in range(4):
                ps = pr_ps.tile([128, 512], F32, tag="pr")
                for kc in range(2):
                    nc.tensor.matmul(
                        ps[:],
                        lhsT=wp_t[kc][:, 128 * mc:128 * (mc + 1)],
                        rhs=pin_sb[kc][:, 512 * pc:512 * (pc + 1)],
                        start=(kc == 0), stop=(kc == 1),
                    )
                nc.vector.tensor_scalar_add(
                    out_sb[mc][:, 512 * pc:512 * (pc + 1)], ps[:], btot_t[mc])
                nc.sync.dma_start(
                    out_ext[128 * mc:128 * (mc + 1), 512 * pc:512 * (pc + 1)],
                    out_sb[mc][:, 512 * pc:512 * (pc + 1)])
        pr_ctx.__exit__(None, None, None)

    nc.finalize()
    return nc


_GRAPH = None


def kernel(**inputs):
    global _GRAPH
    inputs = {k: np.asarray(v, np.float32) for k, v in inputs.items()}
    x = inputs["x"]

    def fold(g, b, m, v):
        inv = g / np.sqrt(v + EPS)
        return inv, b - m * inv

    sq, bq = fold(inputs["qkv_g"], inputs["qkv_b"], inputs["qkv_m"], inputs["qkv_v"])
    spe, bpe = fold(inputs["pe_g"], inputs["pe_b"], inputs["pe_m"], inputs["pe_v"])
    sp, bp = fold(inputs["proj_g"], inputs["proj_b"], inputs["proj_m"], inputs["proj_v"])

    wqkv = np.asarray(inputs["qkv_w"], np.float32)[:, :, 0, 0] * sq[:, None]  # [768,256]
    bqkv = np.asarray(bq, np.float32)
    # permute rows to head-major [Q(256); K(256); V(256)]
    perm = np.empty(3 * C, np.int64)
    for h in range(NH):
        for t in range(3):
            for d in range(HD):
                perm[t * C + HD * h + d] = 3 * HD * h + HD * t + d
    wqkv = wqkv[perm]
    bqkv = bqkv[perm]
    scale = HD ** -0.5
    wqkv[:C] *= scale
    bqkv[:C] *= scale
    b_v = bqkv[2 * C:].copy()

    wpe = np.asarray(inputs["pe_w"], np.float32)[:, 0].reshape(C, 49) * spe[:, None]
    wproj = np.asarray(inputs["proj_w"], np.float32)[:, :, 0, 0] * sp[:, None]
    btot = bp + wproj @ (b_v + bpe)

    wqkv_T = np.ascontiguousarray(wqkv.T).astype(BF16NP)          # [256, 768]
    wproj_T = np.ascontiguousarray(wproj.T).astype(BF16NP)        # [256, 256]
    bqkv_c = np.ascontiguousarray(bqkv[:, None]).astype(np.float32)
    btot_c = np.ascontiguousarray(btot[:, None]).astype(np.float32)
    wpe_c = np.ascontiguousarray(wpe).astype(np.float32)

    wdiag4 = np.zeros((2, NPE, 128, 128), np.float32)
    for ct in range(2):
        for di, (dy, dx) in enumerate(PE_TAPS):
            t = (dy + 3) * 7 + (dx + 3)
            np.fill_diagonal(wdiag4[ct, di], wpe[128 * ct:128 * (ct + 1), t])
    wdiag = np.ascontiguousarray(
        wdiag4.transpose(2, 0, 1, 3).reshape(128, 2 * NPE * 128)).astype(BF16NP)

    xp = np.zeros((B, C, H + 2 * HALO, W), np.float32)
    xp[:, :, HALO:HALO + H] = x

    in_maps = []
    for i in range(NCORES):
        b, r0 = i // 2, 32 * (i % 2)
        slab = xp[b, :, r0:r0 + SLAB_ROWS, :].reshape(C, PX)
        vmask = np.zeros((1, PX), np.float32)
        vr = np.zeros(SLAB_ROWS, np.float32)
        if i % 2 == 0:
            vr[HALO:] = 1.0          # slab rows 0-2 are outside the image
        else:
            vr[:SLAB_ROWS - HALO] = 1.0
        vmask[0] = np.repeat(vr, W)
        in_maps.append({
            "x": slab.astype(BF16NP),
            "vmask": vmask.astype(BF16NP),
            "wdiag": wdiag,
            "wqkv": wqkv_T,
            "bqkv": bqkv_c,
            "wpe": wpe_c,
            "bqkvr": np.ascontiguousarray(bqkv[None, :]).astype(BF16NP),
            "wproj": wproj_T,
            "btot": btot_c,
        })

    if _GRAPH is None:
        _GRAPH = _build_graph()

    trace = os.environ.get("BASS_KERNEL_TRACE") == "1"
    res = run_bass_kernel_spmd(_GRAPH, in_maps, list(range(NCORES)), trace=trace)
    LAST_EXEC_NS[0] = res.exec_time_ns
    LAST_RESULTS[0] = res.results[0]

    out = np.empty((B, C, H, W), np.float32)
    for i in range(NCORES):
        b, r0 = i // 2, 32 * (i % 2)
        out[b, :, r0:r0 + 32, :] = np.asarray(
            res.results[i]["out"], np.float32).reshape(C, 32, W)
    return out

